# revision 34
# baseline (speedup 1.0000x reference)
"""Trainium2 Bass kernel for the patch-correlation + softmax + flow-regression module.

Math: for each batch, match[k,q] = sum_{s in 3x3} <f2n[k+s], f1n[q+s]> where f1n/f2n are
channel-L2-normalized features. flow = softmax_k(10*match) regressed against source coords.

Kernel strategy (per core = one (batch, query-half); 8 cores = 4 batches x 2 halves):
  - Host precomputes the (tiny) channel L2 normalization, scales by 32 and quantizes to
    fp8(e4m3); the device runs the fused correlation+softmax+regression at fp8/bf16.
  - k laid out padded: k' = ki*50 + kj (kj in [0,50), cols 48/49 zero). 24 chunks of 100 rows
    (2 image rows per chunk) so +-1 diagonal shifts never cross useful chunk boundaries.
  - The row-shift (s1) part of the 3x3 patch sum folds into 3 PSUM-accumulated DoubleRow
    fp8 matmuls per (chunk, query-block) with column-shifted operands (contract 256 = both
    channel halves per instruction = 2x bf16 throughput). Operand pair layout: [128, 2, W]
    tiles, pair stride % 16 == 0. j-inner: per (chunk, s1) weight all 3 query blocks are
    streamed back-to-back; V PSUM is a 3-bank tile [128, 3, 512(stride)] per chunk.
  - +-1 diagonal shifts (s2) via exp(V0+Vp+Vm) = exp(V)*shift(exp V)*shift(exp V):
    one scalar Exp over the whole chunk window, 2 full-row partition-shifted SBUF->SBUF
    DMA copies (DMA is the only engine that can shift partitions; column shifts live in
    the DVE read APs), 2 DVE multiplies. The missing halo terms land on zero-pad
    rows/cols where exp(0)=1 keeps the product exact (Em row 0 is preset to 1.0).
  - regression: 3 col-tiled matmuls (out partitions 32j..32j+2 of one PSUM bank, run
    concurrently in the PE array) per chunk, deferred LAG chunks behind the V matmuls so
    the PE never waits on the exp/DMA/mul chain.
  - exp scale 10/1024 un-does the 32x32 feature scaling and applies the softmax x10.
  - Final division + coordinate subtraction on host (tiny: 3x2304 per batch).
"""

from collections import deque

import numpy as np

import concourse.bacc as bacc
import concourse.mybir as mybir
import concourse.tile as tile
from concourse.bass_utils import run_bass_kernel_spmd

F32 = mybir.dt.float32
BF16 = mybir.dt.bfloat16
FP8 = mybir.dt.float8e4
AF = mybir.ActivationFunctionType
DR = mybir.MatmulPerfMode.DoubleRow

H = W = 48
C = 256
HW = H * W
WP = 50              # padded image-row width
KP = H * WP          # 2400 padded k extent
GK = 64              # zero guard cols on each side of feature buffers
QWIN = 26            # f1 window image rows (24 + 1 halo each side)
F1W = QWIN * WP      # 1300
F1G2 = 76            # right guard of f1b so the pair stride is 16B-aligned
F2X = GK + KP + GK   # 2528 (% 16 == 0)
F1X = GK + F1W + F1G2  # 1440 (% 16 == 0)
NCH = 24             # k chunks of 100 rows (2 image rows each)
NBLK = 3             # q blocks per core
QB = 8 * WP          # padded cols per q block (8 image rows)
FS = 32.0            # feature scale into fp8
SC = 10.0 / (FS * FS)  # exp scale: softmax x10 folded with fp8 scaling
LAG = 6              # chunks of deferral for the regression matmuls
NEB = 4              # rotation depth of the shifted-E buffers (pairs)
NSOLO = 2            # trailing chunks processed solo (shorter tail chain)
NWARM = 24           # dummy PE warmup matmuls (HAM un-throttles after ~3.4us)

N_CORES = 8
_CACHE = {}

LAST_EXEC_NS = None
TRACE = False


def _build_nc():
    nc = bacc.Bacc("TRN2", target_bir_lowering=False, debug=False, num_devices=N_CORES)

    f2_in = nc.dram_tensor("f2", [C, KP], FP8, kind="ExternalInput")
    f1_in = nc.dram_tensor("f1", [C, F1W], FP8, kind="ExternalInput")
    wsw_in = nc.dram_tensor("wsw", [128, 3 * NCH], BF16, kind="ExternalInput")
    out_dram = nc.dram_tensor("out", [3, NBLK * QB], F32, kind="ExternalOutput")

    with tile.TileContext(nc) as tc:
        with (
            tc.tile_pool(name="const", bufs=1) as const_pool,
            tc.tile_pool(name="fbuf", bufs=1) as fbuf_pool,
            tc.tile_pool(name="vps", bufs=2, space="PSUM") as v_psum,
            tc.tile_pool(name="wsps", bufs=1, space="PSUM") as ws_psum,
            tc.tile_pool(name="et", bufs=4) as e_pool,
            tc.tile_pool(name="tm", bufs=4) as tm_pool,
            tc.tile_pool(name="me", bufs=6) as me_pool,
            tc.tile_pool(name="mes", bufs=2) as mes_pool,
        ):
            f2b = fbuf_pool.tile([128, 2, F2X], FP8, name="f2b", tag="f2b")
            f1b = fbuf_pool.tile([128, 2, F1X], FP8, name="f1b", tag="f1b")
            # warmup operand memset goes first on the vector queue so the
            # HAM-warmup matmuls can start right after the preamble
            wmt = const_pool.tile([128, 512], BF16)
            nc.vector.memset(wmt[:, :], 0.0)
            # feature loads next so the wires start immediately. f1 first
            # (every chunk contracts against the whole f1 window); f2 split
            # so chunk 0 only waits for its leading columns.
            for src, dst, pieces in (
                (f1_in, f1b, [(0, F1W)]),
                (f2_in, f2b, [(0, 400), (400, KP)]),
            ):
                for o, e in pieces:
                    for i in range(2):
                        dq = nc.sync if i == 0 else nc.gpsimd
                        dq.dma_start(out=dst[:, i, GK + o:GK + e],
                                     in_=src[i * 128:(i + 1) * 128, o:e])
            wsw_t = const_pool.tile([128, 3 * NCH], BF16)
            nc.gpsimd.dma_start(out=wsw_t[:, :], in_=wsw_in[:, :])
            outb = const_pool.tile([128, QB], F32)

            for i in range(2):
                nc.vector.memset(f2b[:, i, 0:GK], 0.0)
                nc.vector.memset(f2b[:, i, GK + KP:F2X], 0.0)
                nc.vector.memset(f1b[:, i, 0:GK], 0.0)
                nc.vector.memset(f1b[:, i, GK + F1W:F1X], 0.0)
            # rotating diag-shift buffers (paired chunks: 6 = 2x3 blocks wide);
            # Em row 0 stays 1.0 forever (the k'-1 halo row of every chunk is
            # a zero-pad row: exp(0) = 1). Both are full-row partition-shifted
            # copies of E (one aligned run per partition); the +-column shifts
            # live in the DVE read APs.
            ep_bufs = [fbuf_pool.tile([128, 2 * NBLK, 402], BF16, name=f"ep{i}", tag=f"ep{i}")
                       for i in range(NEB)]
            em_bufs = [fbuf_pool.tile([128, 2 * NBLK, 402], BF16, name=f"em{i}", tag=f"em{i}")
                       for i in range(NEB)]
            for i in range(NEB):
                nc.vector.memset(em_bufs[i][:, :, :], 1.0)

            # solo-tail buffers (per-chunk postprocessing for the last chunks
            # keeps the end-of-kernel chain short)
            sm_bufs = [fbuf_pool.tile([128, NBLK, 402], BF16, name=f"sm{i}", tag=f"sm{i}")
                       for i in range(NSOLO)]
            for i in range(NSOLO):
                nc.vector.memset(sm_bufs[i][:, :, :], 1.0)
            sp_bufs = [fbuf_pool.tile([128, NBLK, 402], BF16, name=f"sp{i}", tag=f"sp{i}")
                       for i in range(NSOLO)]

            # Main loop: per chunk of 100 k'-rows, 3 DoubleRow matmuls per s1
            # cover all 3 query blocks (j-inner). Chunks are PAIRED for the
            # softmax stage: one Exp per chunk into half of a shared E tile,
            # then per pair 2 diagonal-shift DMAs, 2 DVE products (halves the
            # per-chunk instruction+semaphore overhead on sync/gpsimd/vector).
            # 3 col-tiled regression matmuls per chunk, deferred LAG chunks.
            wsps = ws_psum.tile([128, QB], F32, name="wsps", tag="wsps")
            # dependency-free warmup matmuls fill the input-load dead time so
            # the HAM clock gate opens (1.2 -> 2.4 GHz) before chunk 0; their
            # garbage output is cleared by the first regression matmul's
            # start=True. ~3.4us of sustained PE busy is needed to un-throttle.
            for _ in range(NWARM):
                nc.tensor.matmul(wsps[:, 0:QB], lhsT=wmt[:, 0:128],
                                 rhs=wmt[:, 0:QB], start=True, stop=True,
                                 skip_group_check=True)
            pend = deque()

            def flush_reg():
                c, me_aps = pend.popleft()
                for j in range(NBLK):
                    nc.tensor.matmul(
                        wsps[32 * j:32 * j + 3, :],
                        lhsT=wsw_t[0:100, 3 * c:3 * c + 3],
                        rhs=me_aps[j],
                        start=(c == 0), stop=(c == NCH - 1),
                        skip_group_check=True,
                    )

            def do_mms(c):
                V = v_psum.tile([128, NBLK, 512], F32, name="V", tag="V")
                for s1 in (-1, 0, 1):
                    w0 = GK + 100 * c + 50 * s1
                    for j in range(NBLK):
                        r0 = GK + (1 + 8 * j) * WP - 1 + 50 * s1
                        nc.tensor.matmul(
                            V[0:101, j, 0:402],
                            lhsT=f2b[:, 0:2, w0:w0 + 101],
                            rhs=f1b[:, 0:2, r0:r0 + 402],
                            start=(s1 == -1), stop=(s1 == 1),
                            skip_group_check=True, perf_mode=DR,
                        )
                return V

            for pp in range((NCH - NSOLO) // 2):
                E2 = e_pool.tile([128, 2 * NBLK, 402], BF16, name="E2", tag="E2")
                for ph in range(2):
                    c = 2 * pp + ph
                    V = do_mms(c)
                    nc.scalar.activation(E2[0:101, 3 * ph:3 * ph + 3, :],
                                         V[0:101, :, 0:402], AF.Exp, scale=SC)
                    if len(pend) > LAG:
                        flush_reg()
                ep = ep_bufs[pp % NEB]
                em = em_bufs[pp % NEB]
                nc.sync.dma_start(out=ep[0:100, :, :], in_=E2[1:101, :, :])
                nc.sync.dma_start(out=em[1:101, :, :], in_=E2[0:100, :, :])
                tm = tm_pool.tile([128, 2 * NBLK, QB], BF16, name="tm", tag="tm")
                nc.vector.tensor_mul(tm[0:100, :, :], E2[0:100, :, 1:401],
                                     ep[0:100, :, 2:402])
                me = me_pool.tile([128, 2 * NBLK, QB], BF16, name="me", tag="me")
                nc.vector.tensor_mul(me[0:100, :, :], tm[0:100, :, :],
                                     em[0:100, :, 0:400])
                for ph in range(2):
                    pend.append((2 * pp + ph,
                                 [me[0:100, NBLK * ph + j, :] for j in range(NBLK)]))
            for si in range(NSOLO):
                c = NCH - NSOLO + si
                V = do_mms(c)
                # per-block postprocessing: each block's exp can start as soon
                # as its own PSUM accumulation group stops, and the chain
                # stages pipeline across blocks (short end-of-kernel drain)
                Es = e_pool.tile([128, NBLK, 402], BF16, name="Es", tag="Es")
                nc.scalar.activation(Es[0:101, :, :], V[0:101, :, 0:402],
                                     AF.Exp, scale=SC)
                eps = sp_bufs[si]
                ems = sm_bufs[si]
                nc.sync.dma_start(out=eps[0:100, :, :], in_=Es[1:101, :, :])
                nc.gpsimd.dma_start(out=ems[1:101, :, :], in_=Es[0:100, :, :])
                tms = tm_pool.tile([128, NBLK, QB], BF16, name="tms", tag="tms")
                nc.vector.tensor_mul(tms[0:100, :, :], Es[0:100, :, 1:401],
                                     eps[0:100, :, 2:402])
                mes = mes_pool.tile([128, NBLK, QB], BF16, name="mes", tag="mes")
                nc.vector.tensor_mul(mes[0:100, :, :], tms[0:100, :, :],
                                     ems[0:100, :, 0:400])
                pend.append((c, [mes[0:100, j, :] for j in range(NBLK)]))
                if len(pend) > LAG:
                    flush_reg()
            while pend:
                flush_reg()
            dma_eng = [nc.sync, nc.gpsimd, nc.sync]
            for j in range(NBLK):
                nc.vector.tensor_copy(outb[32 * j:32 * j + 3, :],
                                      wsps[32 * j:32 * j + 3, :])
                dma_eng[j].dma_start(out=out_dram[:, QB * j:QB * (j + 1)],
                                     in_=outb[32 * j:32 * j + 3, :])

    nc.compile()
    return nc


def _pad_rows(x2d):
    # [C, R*48] -> [C, R*50] zero-padding cols 48,49 of each image row
    rows = x2d.shape[1] // W
    out = np.zeros((x2d.shape[0], rows * WP), np.float32)
    out.reshape(x2d.shape[0], rows, WP)[:, :, :W] = x2d.reshape(x2d.shape[0], rows, W)
    return out


def _ws_weights():
    wsw = np.zeros((128, 3 * NCH), np.float32)
    for c in range(NCH):
        kp = 100 * c + np.arange(128)
        ki, kj = kp // WP, kp % WP
        valid = (kp < KP) & (kj < 48) & (np.arange(128) < 100)
        wsw[:, 3 * c + 0] = np.where(valid, ki.astype(np.float32), 0.0)
        wsw[:, 3 * c + 1] = np.where(valid, kj.astype(np.float32), 0.0)
        wsw[:, 3 * c + 2] = np.where(valid, 1.0, 0.0)
    return wsw


def _maybe_enable_trace():
    """Register the axon NTFF profiling hook if available (test-time only)."""
    try:
        import sys
        import types
        if "antenv.axon_hooks" not in sys.modules:
            mod = types.ModuleType("antenv.axon_hooks")
            holder = [None]
            mod.set_axon_ntff_profile_hook = lambda h: holder.__setitem__(0, h)
            mod.get_axon_ntff_profile_hook = lambda: holder[0]
            sys.modules["antenv.axon_hooks"] = mod
        from trn_agent_boot.trn_boot import _ntff_profile_via_ctypes
        sys.modules["antenv.axon_hooks"].set_axon_ntff_profile_hook(
            _ntff_profile_via_ctypes("/opt/axon/libaxon_pjrt.so")
        )
        return True
    except Exception:
        return False


def kernel(feature_1, feature_2):
    global LAST_EXEC_NS
    import ml_dtypes
    f1 = np.asarray(feature_1, dtype=np.float32)
    f2 = np.asarray(feature_2, dtype=np.float32)
    B = f1.shape[0]
    assert f1.shape == (B, C, H, W) and f2.shape == (B, C, H, W)

    if "nc" not in _CACHE:
        _CACHE["nc"] = _build_nc()
    nc = _CACHE["nc"]

    # channel L2 norm on host (tiny), scale 32, quantize e4m3
    def _norm8(x):  # [B, C, HW] fp32 -> fp8
        n = np.sqrt((x * x).sum(axis=1, keepdims=True))
        return (x * (FS / np.maximum(n, 1e-12))).astype(ml_dtypes.float8_e4m3fn)

    f1n = _norm8(f1.reshape(B, C, HW)).astype(np.float32)
    f2n = _norm8(f2.reshape(B, C, HW)).astype(np.float32)

    wsw = _ws_weights().astype(ml_dtypes.bfloat16)
    e4 = ml_dtypes.float8_e4m3fn
    in_maps = []
    for core in range(N_CORES):
        b, half = divmod(core, 2)
        b = b % B
        f2pad = _pad_rows(f2n[b]).astype(e4)
        qi0 = 24 * half
        win = np.zeros((C, QWIN, W), np.float32)
        lo = max(0, qi0 - 1)
        hi = min(H, qi0 + QWIN - 1)
        win[:, lo - (qi0 - 1):hi - (qi0 - 1)] = f1n[b].reshape(C, H, W)[:, lo:hi]
        f1win = _pad_rows(win.reshape(C, QWIN * W)).astype(e4)
        in_maps.append({"f2": f2pad, "f1": f1win, "wsw": wsw})

    trace = TRACE and _maybe_enable_trace()
    res = run_bass_kernel_spmd(nc, in_maps, list(range(N_CORES)), trace=trace)
    LAST_EXEC_NS = res.exec_time_ns

    out = np.zeros((B, 2, H, W), np.float32)
    qj = np.arange(W, dtype=np.float32)[None, :]
    for core in range(N_CORES):
        b, half = divmod(core, 2)
        b = b % B
        o = np.asarray(res.results[core]["out"]).reshape(3, QROWS_ := 24, WP)[:, :, :W]
        eh = o[0] / o[2]
        ew = o[1] / o[2]
        qi0 = 24 * half
        qi = (qi0 + np.arange(QROWS_, dtype=np.float32))[:, None]
        out[b, 0, qi0:qi0 + QROWS_] = ew - qj
        out[b, 1, qi0:qi0 + QROWS_] = eh - qi
    return out
